# revision 49
# baseline (speedup 1.0000x reference)
"""BFLinear (block-floating-point quantized linear) Trainium2 kernel, v3.

Computes: out = bf_quant(bf_quant(x) @ bf_quant(W).T + 2*b)
where bf_quant quantizes groups of 32 along the last axis to a shared
power-of-two exponent with 8 mantissa bits (values = int8 * 2^(e-7)).

Distribution over 8 NeuronCores (SPMD, one identical program):
  - batch dim of x sharded 8 ways (1024 rows/core)
  - W quantization: rank r quantizes + PE-transposes W rows of output
    block r (contribution, AllGathered), and EVERY rank additionally
    quantizes block 7 locally from identical data ("w_sl2").  The
    matmul sweep processes the local block first (zero collective
    dependency), then gathered ranges 0..6.  Rank 7's gathered range
    is never read.  Uniform program, per-core data.
  - a tiny warmup AllGather runs first (absorbs launch skew +
    collective init); the real AllGather is triggered as soon as the
    contribution block is stored, and completes under the local
    matmul work (x-quant + block-7 matmul).
  - matmul sweep: slab-outer, x-tile middle, k innermost: 32
    consecutive matmuls accumulate into one PSUM bank; weight slabs
    are SBUF-resident with one-ahead prefetch.  Measured rate is the
    power-throttled PE ceiling (K=13/16), so the sweep is PE-bound.
  - quantization runs on half-width tiles ([128, 2048]) for pipeline
    latency; round+clamp is a single scalar-engine saturating int8
    convert, dequant is one DVE multiply (int8 x scale -> bf16).

Quantization math (matching jnp semantics):
  m     = max |x| over each group of 32          (abs-max reduce)
  scale = 2^(floor(log2 m) - 7)                  (exponent-field bit math)
  inv   = 1/scale                                (bit math, exact)
  i8    = sat_int8(rne(x*inv))                   (ACT convert)
  q     = i8 * scale                             (exact in bf16)
"""

import numpy as np

# full-problem dimensions (hardcoded per harness contract)
B_FULL = 8192
IN_FULL = 4096
OUT_FULL = 4096
NCORES = 8

P = 128
SZ = 32
NB = 512                      # output column block width (= OUT/NCORES)
HW = 2048                     # quant half-tile width
LOCAL_BLK = NCORES - 1        # the block every core quantizes locally


def build_nc(b_sh=B_FULL // NCORES, in_dim=IN_FULL, out_dim=OUT_FULL,
             ncores=NCORES):
    """Build the SPMD Bass program (identical on every core; data differs)."""
    import concourse.mybir as mybir
    import concourse.tile as tile
    from concourse import bacc

    F32 = mybir.dt.float32
    BF16 = mybir.dt.bfloat16
    I32 = mybir.dt.int32
    I8 = mybir.dt.int8
    ALU = mybir.AluOpType
    AX = mybir.AxisListType
    AF = mybir.ActivationFunctionType

    w_sl = out_dim // ncores          # W rows per block
    kc = in_dim // P                  # 128-wide contraction chunks
    n_xt = b_sh // P                  # x row tiles
    n_wt = w_sl // P                  # W row tiles per block
    kh = HW // P                      # k-chunks per half tile (16)
    assert w_sl == NB and in_dim == 2 * HW

    nc = bacc.Bacc("TRN2", target_bir_lowering=False, debug=False,
                   num_devices=ncores)

    x_sh = nc.dram_tensor("x_sh", [b_sh, in_dim], F32, kind="ExternalInput")
    w_sl_t = nc.dram_tensor("w_sl", [w_sl, in_dim], F32, kind="ExternalInput")
    w_sl2_t = nc.dram_tensor("w_sl2", [w_sl, in_dim], F32,
                             kind="ExternalInput")
    b2_rep = nc.dram_tensor("b2_rep", [P, out_dim], F32, kind="ExternalInput")
    ident_in = nc.dram_tensor("ident", [P, P], BF16, kind="ExternalInput")
    out_sh = nc.dram_tensor("out_sh", [b_sh, out_dim], F32,
                            kind="ExternalOutput")

    wqt_loc = nc.dram_tensor("wqt_loc", [in_dim, NB], BF16)
    # the weight AllGather is split into two k-half collectives so the
    # sweep's first gathered block can start on its first contraction
    # half ~75us before the full gather completes
    KA = kc // 2
    HKA = KA * P
    HKB = in_dim - HKA
    wq_ag_a = nc.dram_tensor("wq_ag_a", [ncores * HKA, NB], BF16,
                             addr_space="Shared")
    wq_ag_b = nc.dram_tensor("wq_ag_b", [ncores * HKB, NB], BF16,
                             addr_space="Shared")
    # warmup-collective scratch: contents never read, only the rendezvous
    # side effect matters (collectives cannot touch IO tensors)
    warm_src = nc.dram_tensor("warm_src", [P, 8], BF16)
    warm_ag = nc.dram_tensor("warm_ag", [ncores * P, 8], BF16,
                             addr_space="Shared")

    with tile.TileContext(nc) as tc:
        from contextlib import ExitStack
        with ExitStack() as ctx:
            qpool = ctx.enter_context(tc.tile_pool(name="qpool", bufs=3))
            spool = ctx.enter_context(tc.tile_pool(name="spool", bufs=3))
            big = ctx.enter_context(tc.tile_pool(name="big", bufs=1))
            wpool = ctx.enter_context(tc.tile_pool(name="wpool", bufs=2))
            opool = ctx.enter_context(tc.tile_pool(name="opool", bufs=3))
            mm_pool = ctx.enter_context(
                tc.tile_pool(name="mmp", bufs=2, space="PSUM"))

            ident = big.tile([P, P], BF16, tag="ident")
            nc.scalar.dma_start(ident[:], ident_in.ap())

            # tiny warmup collective (gpsimd carries only the collectives
            # and post-AG drain multiplies, so nothing local queues behind
            # a collective wait)
            nc.gpsimd.collective_compute(
                "AllGather", ALU.bypass,
                replica_groups=[list(range(ncores))],
                ins=[warm_src.ap().opt()],
                outs=[warm_ag.ap().opt()])

            # ---- quant: half-tiles, phase-split over a pair --------------
            def q_half(src, row, col, tag, queue):
                """Load + start quant chain for one [P, HW] half tile.
                Returns (i8, scale) for the dequant step."""
                xt = qpool.tile([P, HW], F32, tag="xt", name=f"xt_{tag}")
                queue.dma_start(xt[:], src.ap()[row:row + P, col:col + HW])
                return xt

            def q_scales(xt, tag):
                g = HW // SZ
                x3 = xt.rearrange("p (g s) -> p g s", s=SZ)
                m = spool.tile([P, g], F32, tag="q_m", name=f"m_{tag}")
                nc.vector.tensor_reduce(m[:], x3, axis=AX.X, op=ALU.max,
                                        apply_absolute_value=True)
                scale = spool.tile([P, g], F32, tag="q_scale",
                                   name=f"scale_{tag}")
                nc.vector.tensor_scalar(
                    scale[:].bitcast(I32), m[:].bitcast(I32),
                    0x7F800000, None, op0=ALU.bitwise_and)
                nc.vector.tensor_scalar(
                    scale[:].bitcast(I32), scale[:].bitcast(I32),
                    7 << 23, None, op0=ALU.subtract)
                inv = spool.tile([P, g], F32, tag="q_inv", name=f"inv_{tag}")
                nc.vector.tensor_scalar(
                    inv[:].bitcast(I32), scale[:].bitcast(I32),
                    -1, None, op0=ALU.bitwise_xor)
                nc.vector.tensor_scalar(
                    inv[:].bitcast(I32), inv[:].bitcast(I32),
                    (254 << 23) + 1, None, op0=ALU.add)
                return scale, inv

            def q_mult(xt, inv, tag, eng):
                # NOT in-place: writing a fresh tile means the xt slot frees
                # right after this op, so the load WAR loop stays within
                # sync->vector instead of round-tripping through scalar.
                g = HW // SZ
                xm = qpool.tile([P, HW], F32, tag="xm", bufs=2,
                                name=f"xm_{tag}")
                eng.tensor_tensor(
                    xm[:].rearrange("p (g s) -> p g s", s=SZ),
                    xt.rearrange("p (g s) -> p g s", s=SZ),
                    inv[:, :, None].to_broadcast([P, g, SZ]), ALU.mult)
                return xm

            def q_int8(xm, tag):
                i8 = qpool.tile([P, HW], I8, tag="i8", bufs=2,
                                name=f"i8_{tag}")
                nc.scalar.activation(i8[:], xm, AF.Copy, bias=0.0, scale=1.0)
                return i8

            def q_deq(i8, scale, tag, eng):
                g = HW // SZ
                q = qpool.tile([P, HW], BF16, tag="q", bufs=2,
                               name=f"q_{tag}")
                eng.tensor_tensor(
                    q[:].rearrange("p (g s) -> p g s", s=SZ),
                    i8.rearrange("p (g s) -> p g s", s=SZ),
                    scale[:, :, None].to_broadcast([P, g, SZ]), ALU.mult)
                return q

            def quant_pair(src, rowcols, tag, queue, eng):
                """Quantize two [P, HW] half tiles, phase-interleaved.
                `eng` runs the two big multiplies: vector for the x phase;
                gpsimd for the W phases (emitted before each collective in
                gpsimd queue order, they run in the rendezvous slack)."""
                xts = [q_half(src, r, c, f"{tag}{i}", queue)
                       for i, (r, c) in enumerate(rowcols)]
                si = [q_scales(xt[:], f"{tag}{i}")
                      for i, xt in enumerate(xts)]
                xms = [q_mult(xts[i][:], si[i][1][:], f"{tag}{i}", eng)
                       for i in range(2)]
                i8s = [q_int8(xms[i][:], f"{tag}{i}") for i in range(2)]
                return [q_deq(i8s[i][:], si[i][0][:], f"{tag}{i}", eng)
                        for i in range(2)]

            # transpose one bf16 q half-tile (16 k-chunks) into `sink`:
            # sink[:, k0+kq*4:(kq+1)*4, cc:cc+P] gets the transposed chunks
            def half_transpose(q, k0, sink, cc, tag, copy_eng):
                for kq in range(kh // 4):
                    tp = mm_pool.tile([P, 4, P], BF16, tag="tp", bufs=3,
                                      name=f"tp_{tag}_{kq}")
                    for t in range(4):
                        k = kq * 4 + t
                        nc.tensor.transpose(tp[:, t, :],
                                            q[:, k * P:(k + 1) * P], ident[:])
                    copy_eng.copy(
                        sink[:, k0 + kq * 4:k0 + (kq + 1) * 4, cc:cc + P],
                        tp[:])

            # ---- W contribution block: quant + transpose + store + AG ----
            # ALL quant-input loads ride the Sync queue (and nothing else
            # does): the tile scheduler orders a queue by modeled readiness,
            # so a queue mixing pre-AG loads with post-AG loads can end up
            # with every pre-AG load stuck behind the collective wait.
            for i in range(n_wt):
                qs = quant_pair(w_sl_t, [(i * P, 0), (i * P, HW)], f"wc{i}",
                                nc.sync, nc.gpsimd)
                wtt = wpool.tile([P, kc, P], BF16, tag="w", name=f"wtt_{i}")
                for h, q in enumerate(qs):
                    half_transpose(q[:], h * kh, wtt, 0, f"wc{i}{h}",
                                   nc.scalar)
                nc.scalar.dma_start(
                    wqt_loc.ap()[:, i * P:(i + 1) * P].rearrange(
                        "(k p) o -> p k o", p=P),
                    wtt[:, :, 0:P])

            # slabL allocated here (filled later by the W-local section) so
            # the wpool slot cycle keeps every slab one-ahead of its use:
            # [wtt0..3, slabL, slab1, slab_2(waits t0 done), slab_3(t1), ..]
            slabL = wpool.tile([P, kc, NB], BF16, tag="w", name="slabL")

            # main AllGather, k-half a.  Between AG_a and AG_b, slab_1's
            # first-half chunk loads slot onto the gpsimd queue: they only
            # need AG_a, and AG_b's transfer window is busy on the CC ring
            # until AG_a finishes anyway, so triggering AG_b after them
            # does not delay its completion.
            nc.gpsimd.collective_compute(
                "AllGather", ALU.bypass,
                replica_groups=[list(range(ncores))],
                ins=[wqt_loc.ap()[0:HKA, :].opt()],
                outs=[wq_ag_a.ap().opt()])
            slab1 = wpool.tile([P, kc, NB], BF16, tag="w", name="slab_1")
            for ch in range(2):
                klo = ch * (kc // 4)
                khi = (ch + 1) * (kc // 4)
                nc.gpsimd.dma_start(
                    slab1[:, klo:khi, :],
                    wq_ag_a.ap()[klo * P:khi * P, :]
                    .rearrange("(k p) o -> p k o", p=P))
            nc.gpsimd.collective_compute(
                "AllGather", ALU.bypass,
                replica_groups=[list(range(ncores))],
                ins=[wqt_loc.ap()[HKA:in_dim, :].opt()],
                outs=[wq_ag_b.ap().opt()])

            # ---- local W block (block 7, identical on every core) --------
            for i in range(n_wt):
                qs = quant_pair(w_sl2_t, [(i * P, 0), (i * P, HW)], f"wl{i}",
                                nc.sync, nc.gpsimd)
                for h, q in enumerate(qs):
                    half_transpose(q[:], h * kh, slabL, i * P, f"wl{i}{h}",
                                   nc.scalar)

            # ---- x quant + transpose into resident xqT -------------------
            xqT = big.tile([P, kc, b_sh], BF16, tag="xqT")
            for bb in range(n_xt):
                qs = quant_pair(x_sh, [(bb * P, 0), (bb * P, HW)], f"x{bb}",
                                nc.sync, nc.vector)
                for h, q in enumerate(qs):
                    half_transpose(q[:], h * kh, xqT, bb * P, f"x{bb}{h}",
                                   nc.scalar)

            # ---- matmul sweep --------------------------------------------
            def drain(ps, bb, jcol, b2s, mult_eng):
                s = opool.tile([P, NB], F32, tag="ds", name=f"s_{jcol}_{bb}")
                nc.scalar.copy(s[:], ps[:])   # releases the PSUM bank
                nc.vector.tensor_tensor(s[:], s[:], b2s, ALU.add)
                g = NB // SZ
                s3 = s[:].rearrange("p (g s) -> p g s", s=SZ)
                m = opool.tile([P, g], F32, tag="o_m", name=f"om_{jcol}_{bb}")
                nc.vector.tensor_reduce(m[:], s3, axis=AX.X, op=ALU.max,
                                        apply_absolute_value=True)
                scale = opool.tile([P, g], F32, tag="o_scale",
                                   name=f"osc_{jcol}_{bb}")
                nc.vector.tensor_scalar(
                    scale[:].bitcast(I32), m[:].bitcast(I32),
                    0x7F800000, None, op0=ALU.bitwise_and)
                nc.vector.tensor_scalar(
                    scale[:].bitcast(I32), scale[:].bitcast(I32),
                    7 << 23, None, op0=ALU.subtract)
                inv = opool.tile([P, g], F32, tag="o_inv",
                                 name=f"oin_{jcol}_{bb}")
                nc.vector.tensor_scalar(
                    inv[:].bitcast(I32), scale[:].bitcast(I32),
                    -1, None, op0=ALU.bitwise_xor)
                nc.vector.tensor_scalar(
                    inv[:].bitcast(I32), inv[:].bitcast(I32),
                    (254 << 23) + 1, None, op0=ALU.add)
                mult_eng.tensor_tensor(
                    s3, s3, inv[:, :, None].to_broadcast([P, g, SZ]),
                    ALU.mult)
                oi8 = opool.tile([P, NB], I8, tag="oi8",
                                 name=f"oi8_{jcol}_{bb}")
                nc.scalar.activation(oi8[:], s[:], AF.Copy, bias=0.0,
                                     scale=1.0)
                oq = opool.tile([P, NB], F32, tag="oq",
                                name=f"oq_{jcol}_{bb}")
                nc.vector.tensor_tensor(
                    oq[:].rearrange("p (g s) -> p g s", s=SZ),
                    oi8[:].rearrange("p (g s) -> p g s", s=SZ),
                    scale[:, :, None].to_broadcast([P, g, SZ]), ALU.mult)
                nc.scalar.dma_start(
                    out_sh.ap()[bb * P:(bb + 1) * P,
                                jcol * NB:(jcol + 1) * NB],
                    oq[:])

            for t in range(ncores):
                if t == 0:
                    slab = slabL
                    jcol = LOCAL_BLK
                else:
                    jcol = t - 1
                    if t == 1:
                        slab = slab1   # piece-a chunks already loading
                        chunks = range(2, 4)
                    else:
                        slab = wpool.tile([P, kc, NB], BF16, tag="w",
                                          name=f"slab_{t}")
                        chunks = range(4)
                    # slab loads ride gpsimd: everything there is post-AG.
                    # chunked DMAs so the first k-chunks land early and
                    # matmuls start before the whole slab arrives.
                    for ch in chunks:
                        klo = ch * (kc // 4)
                        khi = (ch + 1) * (kc // 4)
                        if ch < 2:
                            src, base = wq_ag_a, jcol * HKA + klo * P
                        else:
                            src, base = wq_ag_b, jcol * HKB + (klo - KA) * P
                        nc.gpsimd.dma_start(
                            slab[:, klo:khi, :],
                            src.ap()[base:base + (khi - klo) * P, :]
                            .rearrange("(k p) o -> p k o", p=P))
                b2s = opool.tile([P, NB], F32, tag="b2s", bufs=2,
                                 name=f"b2s_{t}")
                nc.sync.dma_start(
                    b2s[:],
                    b2_rep.ap()[:, jcol * NB:(jcol + 1) * NB])
                for bb in range(n_xt):
                    ps = mm_pool.tile([P, NB], F32, tag="ps", bufs=5,
                                      name=f"ps_{t}_{bb}")
                    for k in range(kc):
                        nc.tensor.matmul(
                            ps[:],
                            lhsT=xqT[:, k, bb * P:(bb + 1) * P],
                            rhs=slab[:, k, :],
                            start=(k == 0), stop=(k == kc - 1),
                            skip_group_check=True)
                    # t=0 drains run pre-AG: keep them off the gpsimd
                    # queue (which is waiting on the AllGather)
                    drain(ps, bb, jcol, b2s[:],
                          nc.vector if t == 0 else nc.gpsimd)

    nc.compile()
    return nc


_NC_CACHE = {}


def _get_nc(key=(B_FULL // NCORES, IN_FULL, OUT_FULL, NCORES)):
    if key not in _NC_CACHE:
        _NC_CACHE[key] = build_nc(*key)
    return _NC_CACHE[key]


def make_in_maps(x, W, b, ncores=NCORES):
    import ml_dtypes
    b_sh = x.shape[0] // ncores
    w_sl = W.shape[0] // ncores
    out_dim = W.shape[0]
    b2 = (2.0 * np.asarray(b, np.float32)).astype(np.float32)
    b2_rep = np.ascontiguousarray(
        np.broadcast_to(b2.reshape(1, out_dim), (P, out_dim)))
    ident = np.eye(P, dtype=ml_dtypes.bfloat16)
    w_local = np.ascontiguousarray(
        W[LOCAL_BLK * w_sl:(LOCAL_BLK + 1) * w_sl])
    return [
        {
            "x_sh": np.ascontiguousarray(x[c * b_sh:(c + 1) * b_sh]),
            "w_sl": np.ascontiguousarray(W[c * w_sl:(c + 1) * w_sl]),
            "w_sl2": w_local,
            "b2_rep": b2_rep,
            "ident": ident,
        }
        for c in range(ncores)
    ]


def kernel(x, W, b):
    from concourse.bass_utils import run_bass_kernel_spmd

    x = np.asarray(x, np.float32)
    W = np.asarray(W, np.float32)
    b = np.asarray(b, np.float32)
    nc = _get_nc()
    in_maps = make_in_maps(x, W, b)
    res = run_bass_kernel_spmd(nc, in_maps, core_ids=list(range(NCORES)))
    return np.concatenate([res.results[c]["out_sh"] for c in range(NCORES)],
                          axis=0)


# revision 51
# speedup vs baseline: 1.1137x; 1.1137x over previous
"""BFLinear (block-floating-point quantized linear) Trainium2 kernel, v3.

Computes: out = bf_quant(bf_quant(x) @ bf_quant(W).T + 2*b)
where bf_quant quantizes groups of 32 along the last axis to a shared
power-of-two exponent with 8 mantissa bits (values = int8 * 2^(e-7)).

Distribution over 8 NeuronCores (SPMD, one identical program):
  - batch dim of x sharded 8 ways (1024 rows/core)
  - W quantization: rank r quantizes + PE-transposes W rows of output
    block r (contribution, AllGathered), and EVERY rank additionally
    quantizes block 7 locally from identical data ("w_sl2").  The
    matmul sweep processes the local block first (zero collective
    dependency), then gathered ranges 0..6.  Rank 7's gathered range
    is never read.  Uniform program, per-core data.
  - a tiny warmup AllGather runs first (absorbs launch skew +
    collective init); the real AllGather is triggered as soon as the
    contribution block is stored, and completes under the local
    matmul work (x-quant + block-7 matmul).
  - matmul sweep: slab-outer, x-tile middle, k innermost: 32
    consecutive matmuls accumulate into one PSUM bank; weight slabs
    are SBUF-resident with one-ahead prefetch.  Measured rate is the
    power-throttled PE ceiling (K=13/16), so the sweep is PE-bound.
  - quantization runs on half-width tiles ([128, 2048]) for pipeline
    latency; round+clamp is a single scalar-engine saturating int8
    convert, dequant is one DVE multiply (int8 x scale -> bf16).

Quantization math (matching jnp semantics):
  m     = max |x| over each group of 32          (abs-max reduce)
  scale = 2^(floor(log2 m) - 7)                  (exponent-field bit math)
  inv   = 1/scale                                (bit math, exact)
  i8    = sat_int8(rne(x*inv))                   (ACT convert)
  q     = i8 * scale                             (exact in bf16)
"""

import numpy as np

# full-problem dimensions (hardcoded per harness contract)
B_FULL = 8192
IN_FULL = 4096
OUT_FULL = 4096
NCORES = 8

P = 128
SZ = 32
NB = 512                      # output column block width (= OUT/NCORES)
HW = 2048                     # quant half-tile width
LOCAL_BLK = NCORES - 1        # the block every core quantizes locally


def build_nc(b_sh=B_FULL // NCORES, in_dim=IN_FULL, out_dim=OUT_FULL,
             ncores=NCORES):
    """Build the SPMD Bass program (identical on every core; data differs)."""
    import concourse.mybir as mybir
    import concourse.tile as tile
    from concourse import bacc

    F32 = mybir.dt.float32
    BF16 = mybir.dt.bfloat16
    I32 = mybir.dt.int32
    I8 = mybir.dt.int8
    ALU = mybir.AluOpType
    AX = mybir.AxisListType
    AF = mybir.ActivationFunctionType

    w_sl = out_dim // ncores          # W rows per block
    kc = in_dim // P                  # 128-wide contraction chunks
    n_xt = b_sh // P                  # x row tiles
    n_wt = w_sl // P                  # W row tiles per block
    kh = HW // P                      # k-chunks per half tile (16)
    assert w_sl == NB and in_dim == 2 * HW

    nc = bacc.Bacc("TRN2", target_bir_lowering=False, debug=False,
                   num_devices=ncores)

    x_sh = nc.dram_tensor("x_sh", [b_sh, in_dim], F32, kind="ExternalInput")
    w_sl_t = nc.dram_tensor("w_sl", [w_sl, in_dim], F32, kind="ExternalInput")
    w_sl2_t = nc.dram_tensor("w_sl2", [w_sl, in_dim], F32,
                             kind="ExternalInput")
    b2_rep = nc.dram_tensor("b2_rep", [P, out_dim], F32, kind="ExternalInput")
    ident_in = nc.dram_tensor("ident", [P, P], BF16, kind="ExternalInput")
    out_sh = nc.dram_tensor("out_sh", [b_sh, out_dim], F32,
                            kind="ExternalOutput")

    wqt_loc = nc.dram_tensor("wqt_loc", [in_dim, NB], BF16)
    # the weight AllGather is split into two k-half collectives so the
    # sweep's first gathered block can start on its first contraction
    # half ~75us before the full gather completes
    KA = kc // 2
    HKA = KA * P
    HKB = in_dim - HKA
    wq_ag_a = nc.dram_tensor("wq_ag_a", [ncores * HKA, NB], BF16,
                             addr_space="Shared")
    wq_ag_b = nc.dram_tensor("wq_ag_b", [ncores * HKB, NB], BF16,
                             addr_space="Shared")
    # warmup-collective scratch: contents never read, only the rendezvous
    # side effect matters (collectives cannot touch IO tensors)
    warm_src = nc.dram_tensor("warm_src", [P, 8], BF16)
    warm_ag = nc.dram_tensor("warm_ag", [ncores * P, 8], BF16,
                             addr_space="Shared")

    with tile.TileContext(nc) as tc:
        from contextlib import ExitStack
        with ExitStack() as ctx:
            qpool = ctx.enter_context(tc.tile_pool(name="qpool", bufs=3))
            spool = ctx.enter_context(tc.tile_pool(name="spool", bufs=3))
            big = ctx.enter_context(tc.tile_pool(name="big", bufs=1))
            wpool = ctx.enter_context(tc.tile_pool(name="wpool", bufs=2))
            opool = ctx.enter_context(tc.tile_pool(name="opool", bufs=3))
            mm_pool = ctx.enter_context(
                tc.tile_pool(name="mmp", bufs=2, space="PSUM"))

            ident = big.tile([P, P], BF16, tag="ident")
            nc.scalar.dma_start(ident[:], ident_in.ap())

            # tiny warmup collective (gpsimd carries only the collectives
            # and post-AG drain multiplies, so nothing local queues behind
            # a collective wait)
            nc.gpsimd.collective_compute(
                "AllGather", ALU.bypass,
                replica_groups=[list(range(ncores))],
                ins=[warm_src.ap().opt()],
                outs=[warm_ag.ap().opt()])

            # ---- quant: half-tiles, phase-split over a pair --------------
            def q_half(src, row, col, tag, queue):
                """Load + start quant chain for one [P, HW] half tile.
                Returns (i8, scale) for the dequant step."""
                xt = qpool.tile([P, HW], F32, tag="xt", name=f"xt_{tag}")
                queue.dma_start(xt[:], src.ap()[row:row + P, col:col + HW])
                return xt

            def q_scales(xt, tag):
                g = HW // SZ
                x3 = xt.rearrange("p (g s) -> p g s", s=SZ)
                m = spool.tile([P, g], F32, tag="q_m", name=f"m_{tag}")
                nc.vector.tensor_reduce(m[:], x3, axis=AX.X, op=ALU.max,
                                        apply_absolute_value=True)
                scale = spool.tile([P, g], F32, tag="q_scale",
                                   name=f"scale_{tag}")
                nc.vector.tensor_scalar(
                    scale[:].bitcast(I32), m[:].bitcast(I32),
                    0x7F800000, None, op0=ALU.bitwise_and)
                nc.vector.tensor_scalar(
                    scale[:].bitcast(I32), scale[:].bitcast(I32),
                    7 << 23, None, op0=ALU.subtract)
                inv = spool.tile([P, g], F32, tag="q_inv", name=f"inv_{tag}")
                nc.vector.tensor_scalar(
                    inv[:].bitcast(I32), scale[:].bitcast(I32),
                    -1, None, op0=ALU.bitwise_xor)
                nc.vector.tensor_scalar(
                    inv[:].bitcast(I32), inv[:].bitcast(I32),
                    (254 << 23) + 1, None, op0=ALU.add)
                return scale, inv

            def q_mult(xt, inv, tag):
                # NOT in-place: writing a fresh tile means the xt slot frees
                # right after this DVE op, so the load WAR loop stays within
                # sync->vector instead of round-tripping through scalar.
                g = HW // SZ
                xm = qpool.tile([P, HW], F32, tag="xm", bufs=2,
                                name=f"xm_{tag}")
                nc.vector.tensor_tensor(
                    xm[:].rearrange("p (g s) -> p g s", s=SZ),
                    xt.rearrange("p (g s) -> p g s", s=SZ),
                    inv[:, :, None].to_broadcast([P, g, SZ]), ALU.mult)
                return xm

            def q_int8(xm, tag):
                i8 = qpool.tile([P, HW], I8, tag="i8", bufs=2,
                                name=f"i8_{tag}")
                nc.scalar.activation(i8[:], xm, AF.Copy, bias=0.0, scale=1.0)
                return i8

            def q_deq(i8, scale, tag):
                g = HW // SZ
                q = qpool.tile([P, HW], BF16, tag="q", bufs=2,
                               name=f"q_{tag}")
                nc.vector.tensor_tensor(
                    q[:].rearrange("p (g s) -> p g s", s=SZ),
                    i8.rearrange("p (g s) -> p g s", s=SZ),
                    scale[:, :, None].to_broadcast([P, g, SZ]), ALU.mult)
                return q

            def quant_pair(src, rowcols, tag, queue):
                """Quantize two [P, HW] half tiles, phase-interleaved.
                Returns the bf16 q half-tiles."""
                xts = [q_half(src, r, c, f"{tag}{i}", queue)
                       for i, (r, c) in enumerate(rowcols)]
                si = [q_scales(xt[:], f"{tag}{i}")
                      for i, xt in enumerate(xts)]
                xms = [q_mult(xts[i][:], si[i][1][:], f"{tag}{i}")
                       for i in range(2)]
                i8s = [q_int8(xms[i][:], f"{tag}{i}") for i in range(2)]
                return [q_deq(i8s[i][:], si[i][0][:], f"{tag}{i}")
                        for i in range(2)]

            # transpose one bf16 q half-tile (16 k-chunks) into `sink`:
            # sink[:, k0+kq*4:(kq+1)*4, cc:cc+P] gets the transposed chunks
            def half_transpose(q, k0, sink, cc, tag, copy_eng):
                for kq in range(kh // 4):
                    tp = mm_pool.tile([P, 4, P], BF16, tag="tp", bufs=3,
                                      name=f"tp_{tag}_{kq}")
                    for t in range(4):
                        k = kq * 4 + t
                        nc.tensor.transpose(tp[:, t, :],
                                            q[:, k * P:(k + 1) * P], ident[:])
                    copy_eng.copy(
                        sink[:, k0 + kq * 4:k0 + (kq + 1) * 4, cc:cc + P],
                        tp[:])

            # ---- W contribution block: quant + transpose + store + AG ----
            # ALL quant-input loads ride the Sync queue (and nothing else
            # does): the tile scheduler orders a queue by modeled readiness,
            # so a queue mixing pre-AG loads with post-AG loads can end up
            # with every pre-AG load stuck behind the collective wait.
            for i in range(n_wt):
                qs = quant_pair(w_sl_t, [(i * P, 0), (i * P, HW)], f"wc{i}",
                                nc.sync)
                wtt = wpool.tile([P, kc, P], BF16, tag="w", name=f"wtt_{i}")
                for h, q in enumerate(qs):
                    half_transpose(q[:], h * kh, wtt, 0, f"wc{i}{h}",
                                   nc.scalar)
                nc.scalar.dma_start(
                    wqt_loc.ap()[:, i * P:(i + 1) * P].rearrange(
                        "(k p) o -> p k o", p=P),
                    wtt[:, :, 0:P])

            # slabL allocated here (filled later by the W-local section) so
            # the wpool slot cycle keeps every slab one-ahead of its use:
            # [wtt0..3, slabL, slab1, slab_2(waits t0 done), slab_3(t1), ..]
            slabL = wpool.tile([P, kc, NB], BF16, tag="w", name="slabL")

            # main AllGather, k-half a.  Between AG_a and AG_b, slab_1's
            # first-half chunk loads slot onto the gpsimd queue: they only
            # need AG_a, and AG_b's transfer window is busy on the CC ring
            # until AG_a finishes anyway, so triggering AG_b after them
            # does not delay its completion.
            nc.gpsimd.collective_compute(
                "AllGather", ALU.bypass,
                replica_groups=[list(range(ncores))],
                ins=[wqt_loc.ap()[0:HKA, :].opt()],
                outs=[wq_ag_a.ap().opt()])
            slab1 = wpool.tile([P, kc, NB], BF16, tag="w", name="slab_1")
            for ch in range(2):
                klo = ch * (kc // 4)
                khi = (ch + 1) * (kc // 4)
                nc.gpsimd.dma_start(
                    slab1[:, klo:khi, :],
                    wq_ag_a.ap()[klo * P:khi * P, :]
                    .rearrange("(k p) o -> p k o", p=P))
            nc.gpsimd.collective_compute(
                "AllGather", ALU.bypass,
                replica_groups=[list(range(ncores))],
                ins=[wqt_loc.ap()[HKA:in_dim, :].opt()],
                outs=[wq_ag_b.ap().opt()])

            # ---- local W block (block 7) + x quant, interleaved ----------
            # x feeds the lhsT of ALL sweep blocks; W-local only block t=0.
            # Weaving the 4 W-local tiles into the first half of the x
            # stream finishes x-quant ~40us earlier (so no transposes leak
            # into the post-gather sweep) while slabL is still complete in
            # time for the local block to run inside the gather window.
            xqT = big.tile([P, kc, b_sh], BF16, tag="xqT")

            def do_x(bb):
                qs = quant_pair(x_sh, [(bb * P, 0), (bb * P, HW)], f"x{bb}",
                                nc.sync)
                for h, q in enumerate(qs):
                    half_transpose(q[:], h * kh, xqT, bb * P, f"x{bb}{h}",
                                   nc.scalar)

            def do_wl(i):
                qs = quant_pair(w_sl2_t, [(i * P, 0), (i * P, HW)], f"wl{i}",
                                nc.sync)
                for h, q in enumerate(qs):
                    half_transpose(q[:], h * kh, slabL, i * P, f"wl{i}{h}",
                                   nc.scalar)

            for i in range(n_wt):
                do_x(i)
                do_wl(i)
            for bb in range(n_wt, n_xt):
                do_x(bb)

            # ---- matmul sweep --------------------------------------------
            def drain(ps, bb, jcol, b2s, mult_eng):
                s = opool.tile([P, NB], F32, tag="ds", name=f"s_{jcol}_{bb}")
                nc.scalar.copy(s[:], ps[:])   # releases the PSUM bank
                nc.vector.tensor_tensor(s[:], s[:], b2s, ALU.add)
                g = NB // SZ
                s3 = s[:].rearrange("p (g s) -> p g s", s=SZ)
                m = opool.tile([P, g], F32, tag="o_m", name=f"om_{jcol}_{bb}")
                nc.vector.tensor_reduce(m[:], s3, axis=AX.X, op=ALU.max,
                                        apply_absolute_value=True)
                scale = opool.tile([P, g], F32, tag="o_scale",
                                   name=f"osc_{jcol}_{bb}")
                nc.vector.tensor_scalar(
                    scale[:].bitcast(I32), m[:].bitcast(I32),
                    0x7F800000, None, op0=ALU.bitwise_and)
                nc.vector.tensor_scalar(
                    scale[:].bitcast(I32), scale[:].bitcast(I32),
                    7 << 23, None, op0=ALU.subtract)
                inv = opool.tile([P, g], F32, tag="o_inv",
                                 name=f"oin_{jcol}_{bb}")
                nc.vector.tensor_scalar(
                    inv[:].bitcast(I32), scale[:].bitcast(I32),
                    -1, None, op0=ALU.bitwise_xor)
                nc.vector.tensor_scalar(
                    inv[:].bitcast(I32), inv[:].bitcast(I32),
                    (254 << 23) + 1, None, op0=ALU.add)
                mult_eng.tensor_tensor(
                    s3, s3, inv[:, :, None].to_broadcast([P, g, SZ]),
                    ALU.mult)
                oi8 = opool.tile([P, NB], I8, tag="oi8",
                                 name=f"oi8_{jcol}_{bb}")
                nc.scalar.activation(oi8[:], s[:], AF.Copy, bias=0.0,
                                     scale=1.0)
                oq = opool.tile([P, NB], F32, tag="oq",
                                name=f"oq_{jcol}_{bb}")
                nc.vector.tensor_tensor(
                    oq[:].rearrange("p (g s) -> p g s", s=SZ),
                    oi8[:].rearrange("p (g s) -> p g s", s=SZ),
                    scale[:, :, None].to_broadcast([P, g, SZ]), ALU.mult)
                nc.scalar.dma_start(
                    out_sh.ap()[bb * P:(bb + 1) * P,
                                jcol * NB:(jcol + 1) * NB],
                    oq[:])

            for t in range(ncores):
                if t == 0:
                    slab = slabL
                    jcol = LOCAL_BLK
                else:
                    jcol = t - 1
                    if t == 1:
                        slab = slab1   # piece-a chunks already loading
                        chunks = range(2, 4)
                    else:
                        slab = wpool.tile([P, kc, NB], BF16, tag="w",
                                          name=f"slab_{t}")
                        chunks = range(4)
                    # slab loads ride gpsimd: everything there is post-AG.
                    # chunked DMAs so the first k-chunks land early and
                    # matmuls start before the whole slab arrives.
                    for ch in chunks:
                        klo = ch * (kc // 4)
                        khi = (ch + 1) * (kc // 4)
                        if ch < 2:
                            src, base = wq_ag_a, jcol * HKA + klo * P
                        else:
                            src, base = wq_ag_b, jcol * HKB + (klo - KA) * P
                        nc.gpsimd.dma_start(
                            slab[:, klo:khi, :],
                            src.ap()[base:base + (khi - klo) * P, :]
                            .rearrange("(k p) o -> p k o", p=P))
                b2s = opool.tile([P, NB], F32, tag="b2s", bufs=2,
                                 name=f"b2s_{t}")
                nc.sync.dma_start(
                    b2s[:],
                    b2_rep.ap()[:, jcol * NB:(jcol + 1) * NB])
                for bb in range(n_xt):
                    ps = mm_pool.tile([P, NB], F32, tag="ps", bufs=5,
                                      name=f"ps_{t}_{bb}")
                    for k in range(kc):
                        nc.tensor.matmul(
                            ps[:],
                            lhsT=xqT[:, k, bb * P:(bb + 1) * P],
                            rhs=slab[:, k, :],
                            start=(k == 0), stop=(k == kc - 1),
                            skip_group_check=True)
                    # t=0 drains run pre-AG: keep them off the gpsimd
                    # queue (which is waiting on the AllGather)
                    drain(ps, bb, jcol, b2s[:],
                          nc.vector if t == 0 else nc.gpsimd)

    nc.compile()
    return nc


_NC_CACHE = {}


def _get_nc(key=(B_FULL // NCORES, IN_FULL, OUT_FULL, NCORES)):
    if key not in _NC_CACHE:
        _NC_CACHE[key] = build_nc(*key)
    return _NC_CACHE[key]


def make_in_maps(x, W, b, ncores=NCORES):
    import ml_dtypes
    b_sh = x.shape[0] // ncores
    w_sl = W.shape[0] // ncores
    out_dim = W.shape[0]
    b2 = (2.0 * np.asarray(b, np.float32)).astype(np.float32)
    b2_rep = np.ascontiguousarray(
        np.broadcast_to(b2.reshape(1, out_dim), (P, out_dim)))
    ident = np.eye(P, dtype=ml_dtypes.bfloat16)
    w_local = np.ascontiguousarray(
        W[LOCAL_BLK * w_sl:(LOCAL_BLK + 1) * w_sl])
    return [
        {
            "x_sh": np.ascontiguousarray(x[c * b_sh:(c + 1) * b_sh]),
            "w_sl": np.ascontiguousarray(W[c * w_sl:(c + 1) * w_sl]),
            "w_sl2": w_local,
            "b2_rep": b2_rep,
            "ident": ident,
        }
        for c in range(ncores)
    ]


def kernel(x, W, b):
    from concourse.bass_utils import run_bass_kernel_spmd

    x = np.asarray(x, np.float32)
    W = np.asarray(W, np.float32)
    b = np.asarray(b, np.float32)
    nc = _get_nc()
    in_maps = make_in_maps(x, W, b)
    res = run_bass_kernel_spmd(nc, in_maps, core_ids=list(range(NCORES)))
    return np.concatenate([res.results[c]["out_sh"] for c in range(NCORES)],
                          axis=0)
